# revision 20
# baseline (speedup 1.0000x reference)
"""Trainium2 Bass kernel for nn_MultiHeadAttention_75737453297867.

Sharding: one head per NeuronCore (8 heads / 8 cores). The reference's
aliased as_strided gather needs a per-core base offset 24576*h into the
flange-padded k/v storage; 24576*h mod 9216 is row-aligned (phi in
{0,48,96}), so three phi-shifted staging variants are built statically
and selection happens through host data alone (stacked conv channels,
host-built conv weights pick the active variant).

v2 pipeline: the softmax-exp stream is split across ACT (exact exp ->
bf16) and DVE (1-instruction Schraudolph exp: i16 = a*s + b bit-cast as
bf16, ~3% sawtooth that washes out in the softmax ratio).  Keys whose
gathered column lands in the zero-pad band (identical across all
channels - 1440 of 5760 per window) are compacted away; their exp(0)=1
denominator contribution is re-added as a constant.  34 key-chunks of
128 remain per window (tail chunk padded with zeroed keys).

The AV matmul is flipped: stationary = exp-score chunk [128k x 128q]
(ldweights), moving = uvT [128k x 13] (col 0 = ones = denominator), so
AV costs 13 PE rows per chunk instead of 384.  Output lands [q, 13] in
PSUM; normalization is a per-partition reciprocal + tensor_scalar mult,
then a PE transpose puts channels back on partitions for the 3x3 output
conv, which streams one q-chunk behind attention.
"""

import sys

import numpy as np

if "/opt/trn_rl_repo" not in sys.path:
    sys.path.insert(0, "/opt/trn_rl_repo")

import concourse.bass as bass
import concourse.tile as tile
from concourse import bacc
from concourse import mybir
from concourse.bass_types import AP

# Problem constants
CIN, COUT, H, W = 64, 64, 128, 48
DM, NH, DPH = 32, 8, 4
Q0, Q1, F0, F1 = 128, 24, 8, 8
M0, M1 = Q0 + 2 * F0, Q1 + 2 * F1          # 144, 40
CH = 144 * 64                              # 9216 flat padded-channel size
DST = 6144                                 # d-stride (Hp*Wp) in flat coords
PHIS = (0, 48, 96)
F32 = mybir.dt.float32
F32R = mybir.dt.float32r
BF16 = mybir.dt.bfloat16
I16 = mybir.dt.int16

# Compacted key layout: per 4-row group of the 144-row window, the keys
# whose flat column (48*m0 + 24*j + m1) mod 64 lands in the zero band
# [0,8)|[56,64) are dropped (all staged channels are zero there).
# runs[(j)] = list of (m0%4, m1_lo, m1_hi) kept.
RUNS = {
    0: [(0, 8, 40), (1, 0, 8), (1, 24, 40), (2, 0, 24), (3, 0, 40)],
    1: [(0, 0, 32), (1, 0, 40), (2, 16, 40), (3, 0, 16), (3, 32, 40)],
}
NKEEP = 4320                               # kept keys per window
NZERO = float(5760 - NKEEP)                # dropped keys -> +exp(0) each
NCH = 34                                   # key chunks of 128 (tail = 96)
UKW = NCH * 128                            # 4352 (32-col zeroed tail)
NQC = 8                                    # q chunks of 16 rows
QROWS = 16
QC = QROWS * Q1                            # 384 queries per (qc, j)
NRND = 17                                  # exp rounds per (qc, j), RPB=2

# Schraudolph exp in bf16-bits domain: i16 = A*s + B, bitcast bf16.
LOG2E = 1.4426950408889634
SCH_A = float(np.float32(128.0 * LOG2E))
SCH_B = float(np.float32(127.0 * 128.0 - 0.057985 * 128.0 + 0.5))


def _run_copies(eng, dst, src_flat, j, g0, g1):
    """Compacted window copies: 5 strided run-copies per window."""
    col = 0
    for p, lo, hi in RUNS[j]:
        ln = hi - lo
        src = AP(tensor=src_flat.tensor,
                 offset=src_flat.offset + 192 * g0 + 48 * p + 24 * j + lo,
                 ap=[src_flat.ap[0], [192, g1 - g0], [1, ln]])
        d = AP(tensor=dst.tensor, offset=dst.offset + 120 * g0 + col,
               ap=[dst.ap[0], [120, g1 - g0], [1, ln]])
        if hasattr(eng, "tensor_copy"):
            eng.tensor_copy(d, src)
        else:
            eng.copy(d, src)
        col += ln


def build_nc():
    nc = bacc.Bacc()

    xp_d = nc.dram_tensor("xp", [128, 130 * 50], F32R, kind="ExternalInput")
    wqkv_d = nc.dram_tensor("wqkv_t", [128, 6 * 48], F32R, kind="ExternalInput")
    b48_d = nc.dram_tensor("b48", [48, 1], F32, kind="ExternalInput")
    wo_d = nc.dram_tensor("wo_t", [12, 9 * 64], F32R, kind="ExternalInput")
    id16_d = nc.dram_tensor("id16", [16, 16], F32, kind="ExternalInput")
    id12_d = nc.dram_tensor("id12", [12, 12], F32R, kind="ExternalInput")
    id128_d = nc.dram_tensor("id128", [128, 128], F32R, kind="ExternalInput")
    out_d = nc.dram_tensor("out", [COUT, H * W], F32, kind="ExternalOutput")

    from contextlib import ExitStack

    with tile.TileContext(nc) as tc, ExitStack() as ctx:
        P = ctx.enter_context(tc.tile_pool(name="persist", bufs=1))
        dram = ctx.enter_context(tc.tile_pool(name="dram", bufs=1, space="DRAM"))
        ctx1 = ctx.enter_context(ExitStack())
        P1 = ctx1.enter_context(tc.tile_pool(name="phase1", bufs=1))

        # ---- input loads (xp split across two DMA lanes). Partitions
        # 64-127 hold x shifted one column left, so one matmul covers the
        # (dy,0)+(dy,1) tap pair with stacked weights (K=128) ----
        wqkv_sb = P.tile([128, 6, 48], F32R, tag="wqkv")
        nc.sync.dma_start(
            out=wqkv_sb, in_=wqkv_d[:, :].rearrange("p (t o) -> p t o", t=6)
        )
        xp_sb = P1.tile([128, 130, 50], F32R, tag="xp")
        nc.sync.dma_start(
            out=xp_sb[:, 0:45, :],
            in_=xp_d[:, 0:2250].rearrange("p (a b) -> p a b", a=45),
        )
        nc.scalar.dma_start(
            out=xp_sb[:, 45:90, :],
            in_=xp_d[:, 2250:4500].rearrange("p (a b) -> p a b", a=45),
        )
        nc.gpsimd.dma_start(
            out=xp_sb[:, 90:130, :],
            in_=xp_d[:, 4500:6500].rearrange("p (a b) -> p a b", a=40),
        )
        b48 = P.tile([48, 1], F32, tag="b48")
        nc.gpsimd.dma_start(out=b48, in_=b48_d[:, :])
        id16 = P.tile([16, 16], F32, tag="id16")
        nc.gpsimd.dma_start(out=id16, in_=id16_d[:, :])
        wo_sb = P.tile([12, 9, 64], F32R, tag="wo")
        nc.gpsimd.dma_start(
            out=wo_sb, in_=wo_d[:, :].rearrange("p (t o) -> p t o", t=9)
        )
        id12 = P.tile([12, 12], F32R, tag="id12")
        nc.gpsimd.dma_start(out=id12, in_=id12_d[:, :])
        id128 = P.tile([128, 128], F32R, tag="id128")
        nc.gpsimd.dma_start(out=id128, in_=id128_d[:, :])

        zero_sb = P1.tile([128, 648], F32, tag="zeros")
        nc.vector.memset(zero_sb, 0.0)
        pzero = P.tile([128, 1], F32, tag="pzero")
        nc.vector.memset(pzero, 0.0)

        # PE p-state warm-up: f32r dummy matmuls (1 cyc/row) keep PE busy
        # through the xp load so the conv starts at full clock (ramp needs
        # 3us of continuous busy; an idle gap resets to mid-clock)
        with tc.tile_pool(name="pwarm", bufs=1, space="PSUM") as pwarm:
            pw = pwarm.tile([1, 288], F32, tag="pw")
            for _ in range(18):
                nc.tensor.matmul(pw, wqkv_sb[0:1, 0, 0:1],
                                 wqkv_sb[0:1, :, :].rearrange("p a b -> p (a b)"),
                                 start=True, stop=True)

        # ---- DRAM staging buffers (3 variants x 3 channels each) ----
        kp_all = dram.tile([9, CH], F32, tag="kp")
        vp_all = dram.tile([9, CH], F32, tag="vp")
        for buf in (kp_all, vp_all):
            dst = AP(tensor=buf.tensor, offset=buf.offset,
                     ap=[[648, 128], [1, 648]])
            nc.sync.dma_start(out=dst, in_=zero_sb[:, :])

        # ---- stacked q/k/v conv: rows 0-11 = q, rows 32-47 = k4 + v12 ----
        # (kv starts at 32: engine PSUM access must be 32-partition aligned)
        q_sb = P.tile([12, 128, 48], F32R, tag="q_sb")
        kv_sb = P1.tile([16, 128, 48], F32, tag="kv_sb")
        with tc.tile_pool(name="psc", bufs=4, space="PSUM") as psc:
            for chv in range(16):
                ps = psc.tile([48, 8, 48], F32, tag="cps")
                for t in range(6):
                    dy, dx = t // 2, (t % 2) * 2
                    rhs = xp_sb[:, 8 * chv + dy : 8 * chv + dy + 8, dx : dx + 48]
                    nc.tensor.matmul(
                        ps[:, :, :], wqkv_sb[:, t, 0:48], rhs,
                        start=(t == 0), stop=(t == 5),
                    )
                nc.vector.tensor_scalar_add(
                    q_sb[:, 8 * chv : 8 * chv + 8, :], ps[0:12, :, :],
                    b48[0:12, 0:1],
                )
                nc.scalar.add(
                    kv_sb[:, 8 * chv : 8 * chv + 8, :], ps[32:48, :, :],
                    b48[32:48, 0:1],
                )

        # ---- transpose k/v to row-major [128 rows, 16 ch, 48 cols] ----
        kv_row = P1.tile([128, 16, 48], F32, tag="kv_row")
        with tc.tile_pool(name="pst", bufs=4, space="PSUM") as pst:
            for x0 in range(0, 48, 3):
                tp = pst.tile([128, 3, 16], F32, tag="tp")
                for x in range(x0, x0 + 3):
                    nc.tensor.matmul(tp[:, x - x0, :], kv_sb[:, :, x],
                                     id16[:, :], start=True, stop=True)
                tpb = tp[:, 0:1, 0:1]
                src = AP(tensor=tpb.tensor, offset=tpb.offset,
                         ap=[tpb.ap[0], [1, 16], [16, 3]])
                nc.vector.tensor_copy(kv_row[:, :, x0 : x0 + 3], src)

        # ---- phi-shifted staging writes into the padded channel images ----
        engs = [nc.sync, nc.gpsimd, nc.scalar]
        ei = 0
        for buf_all, cbase in ((kp_all, lambda v: 0), (vp_all, lambda v: 4 + 4 * v)):
            for v, phi in enumerate(PHIS):
                cb = cbase(v)
                base = buf_all.offset + 3 * v * CH
                if phi == 0:
                    dst = AP(tensor=buf_all.tensor, offset=base + 8 * 64 + 8,
                             ap=[[64, 128], [CH, 3], [1, 48]])
                    engs[ei % 3].dma_start(out=dst, in_=kv_row[0:128, cb : cb + 3, :])
                    ei += 1
                else:
                    n1 = 136 - phi
                    dst1 = AP(tensor=buf_all.tensor, offset=base + 8,
                              ap=[[64, n1], [CH, 3], [1, 48]])
                    engs[ei % 3].dma_start(
                        out=dst1, in_=kv_row[phi - 8 : 128, cb : cb + 3, :])
                    ei += 1
                    n2 = phi - 8
                    dst2 = AP(tensor=buf_all.tensor,
                              offset=base + (152 - phi) * 64 + 8,
                              ap=[[64, n2], [CH, 3], [1, 48]])
                    engs[ei % 3].dma_start(
                        out=dst2, in_=kv_row[0 : phi - 8, cb + 1 : cb + 4, :])
                    ei += 1

        # ---- padded attention-output image; zero only the 1-px border ----
        o_pad = P.tile([12, 130, 50], F32R, tag="opad")
        zb = zero_sb[0:12, 0:1]
        for dst in (o_pad[:, 0, :], o_pad[:, 129, :],
                    o_pad[:, 1:129, 0], o_pad[:, 1:129, 49]):
            n = dst.free_size()
            src = AP(tensor=zb.tensor, offset=zb.offset, ap=[zb.ap[0], [0, n]])
            nc.vector.tensor_copy(dst, src)

        ctx1.close()  # free xp / kv_sb / kv_row / zeros SBUF
        ctx3 = ctx.enter_context(ExitStack())
        uvp = ctx3.enter_context(tc.tile_pool(name="uvp", bufs=2))
        ctx2 = ctx.enter_context(ExitStack())
        P2 = ctx2.enter_context(tc.tile_pool(name="phase2", bufs=1))

        # ---- flat loads. Only [0:6960] is ever read by the window views,
        # so the tiles stop there. Lane plan: uk half 1 first (feeds the
        # j=0 QK chain), then uv halves, then uk half 2 ----
        uk_flat = P2.tile([12, 6960], F32R, tag="uk")
        uv_flat = P2.tile([12, 6960], F32R, tag="uv")

        def load_flat(dst, src_all, lo, hi, eng):
            src = AP(tensor=src_all.tensor, offset=src_all.offset + lo,
                     ap=[[3 * CH, 3], [DST, 4], [1, hi - lo]])
            eng.dma_start(out=dst[:, lo:hi], in_=src.bitcast(F32R))

        load_flat(uk_flat, kp_all, 0, 1740, nc.sync)
        load_flat(uk_flat, kp_all, 1740, 3480, nc.scalar)
        load_flat(uv_flat, vp_all, 0, 3480, nc.gpsimd)
        load_flat(uk_flat, kp_all, 3480, 6960, nc.gpsimd)
        load_flat(uv_flat, vp_all, 3480, 6960, nc.sync)

        # ---- compacted window operands ----
        # ukr[j] [12, 4352] f32r (QK lhsT); tail cols 4320:4352 zeroed.
        # uvr[j] [12, 4352] f32r feeds the uvT transposes, then freed.
        ukr0 = P.tile([12, UKW], F32R, tag="ukr0")
        ukr1 = P.tile([12, UKW], F32R, tag="ukr1")
        ukr = [ukr0, ukr1]
        uvr = []
        for j in range(2):
            uvr_t = uvp.tile([12, UKW], F32R, tag="uvr")
            uvr.append(uvr_t)
        zb12 = pzero[0:12, 0:1]
        zbc = AP(tensor=zb12.tensor, offset=zb12.offset,
                 ap=[zb12.ap[0], [0, UKW - NKEEP]])
        for j in range(2):
            nc.vector.tensor_copy(ukr[j][:, NKEEP:UKW], zbc)
            nc.vector.tensor_copy(uvr[j][:, NKEEP:UKW], zbc)
        # j=0 on ACT/DVE in two group-halves (half 1 of the flat load
        # covers groups 0..18); j=1 on Pool (full load required anyway)
        _run_copies(nc.scalar, ukr[0], uk_flat, 0, 0, 8)
        _run_copies(nc.scalar, ukr[0], uk_flat, 0, 8, 18)
        _run_copies(nc.scalar, ukr[0], uk_flat, 0, 18, 36)
        _run_copies(nc.vector, uvr[0], uv_flat, 0, 0, 18)
        _run_copies(nc.vector, uvr[0], uv_flat, 0, 18, 36)
        _run_copies(nc.gpsimd, ukr[1], uk_flat, 1, 0, 18)
        _run_copies(nc.gpsimd, uvr[1], uv_flat, 1, 0, 18)
        _run_copies(nc.gpsimd, ukr[1], uk_flat, 1, 18, 36)
        _run_copies(nc.gpsimd, uvr[1], uv_flat, 1, 18, 36)

        # ---- uvT[j] [128, 34, 13] bf16: col 0 = ones (denominator); the
        # v-chunk transposes are emitted inside attention slots t0/t1 (uvT[j]
        # is first consumed by the AV burst one slot later) ----
        uvT = []
        for j in range(2):
            t = P.tile([128, NCH, 13], BF16, tag="uvt" + str(j))
            uvT.append(t)
            nc.vector.memset(t[:, :, 0:1], 1.0)
            # fake tail keys (96:128 of chunk 33) must not count
            nc.vector.memset(t[96:128, NCH - 1, 0:1], 0.0)

        ctx2.close()  # free uk_flat / uv_flat SBUF (uvr stays in uvp)
        PL = ctx.enter_context(tc.tile_pool(name="late", bufs=1))
        expool = ctx.enter_context(tc.tile_pool(name="expool", bufs=24))

        # ---- attention: per (qc, j) slot t: 17 QK+exp rounds (RPB=2).
        # exp alternates ACT (exact, bf16 out) / DVE (Schraudolph).
        # The AV burst for slot t-1 is emitted early in slot t; the final
        # conv streams one q-chunk behind. ----
        out_sb = PL.tile([COUT, 128, 48], F32, tag="outsb")
        dma_engs = (nc.sync, nc.gpsimd)
        OUT_DMA = {3: (0, 1536, 0), 7: (1536, 3072, 1), 11: (3072, 4608, 0),
                   12: (4608, 4992, 1), 13: (4992, 5376, 0),
                   14: (5376, 5760, 1), 15: (5760, 6144, 0)}
        den3 = PL.tile([128, 3], F32, tag="den3")
        rec3 = PL.tile([128, 3], F32, tag="rec3")
        nrm = PL.tile([128, 3, 12], F32R, tag="nrm")

        with (
            tc.tile_pool(name="psqk", bufs=4, space="PSUM") as psqk,
        ):
            prev = None          # (exs, ps_av, qc, j) of slot t-1
            ci = 0               # next final-conv chunk

            def alloc_scr():
                s = psqk.tile([128, 2, 512], F32, tag="qk")
                return s

            def av_burst(exs, ps_av, j):
                for sub in range(3):
                    for c in range(NCH):
                        ex = exs[c // 2]
                        nc.tensor.matmul(
                            ps_av[:, sub, :],
                            ex[:, c % 2, 128 * sub : 128 * (sub + 1)],
                            uvT[j][:, c, :],
                            start=(c == 0), stop=(c == NCH - 1),
                            skip_group_check=True)

            def norm_chain(ps_av):
                # DVE: den += nzero, reciprocal, per-partition-scalar mults
                src = AP(tensor=ps_av.tensor, offset=ps_av.offset,
                         ap=[[ps_av.ap[0][0], 128], [13, 3], [1, 1]])
                nc.vector.tensor_scalar(den3, src, NZERO, None,
                                        mybir.AluOpType.add)
                nc.vector.reciprocal(rec3, den3)
                for sub in range(3):
                    nc.vector.tensor_scalar(
                        nrm[:, sub, :], ps_av[:, sub, 1:13],
                        rec3[:, sub : sub + 1], None, mybir.AluOpType.mult)

            def norm_out(scr, qc, j):
                # PE transposes into the scr tile (bank0, after the av cols),
                # ACT copy into o_pad. Deferred to r3 so the PE stream never
                # blocks on the DVE norm chain.
                for sub in range(3):
                    tps = AP(tensor=scr.tensor,
                             offset=scr.offset + 40 + 128 * sub,
                             ap=[[scr.ap[0][0], 12], [1, 128]]).bitcast(F32R)
                    nc.tensor.transpose(tps, nrm[:, sub, :], id128[:, :])
                dst = o_pad[:, 1 + QROWS * qc : 1 + QROWS * (qc + 1),
                            1 + 24 * j : 25 + 24 * j]
                src_tp = AP(tensor=scr.tensor, offset=scr.offset + 40,
                            ap=[[scr.ap[0][0], 12], [24, QROWS], [1, 24]]
                            ).bitcast(F32R)
                nc.scalar.copy(dst, src_tp)

            def conv_taps(cv, t0_):
                scr, c = cv
                ps = AP(tensor=scr.tensor, offset=scr.offset,
                        ap=[[scr.ap[0][0], COUT], [48, 8], [1, 48]])
                for t9 in range(t0_, t0_ + 3):
                    dy, dx = t9 // 3, t9 % 3
                    rhs = o_pad[:, 8 * c + dy : 8 * c + dy + 8, dx : dx + 48]
                    nc.tensor.matmul(ps[:, :, :], wo_sb[:, t9, :], rhs,
                                     start=(t9 == 0), stop=(t9 == 8))

            def conv_out(cv):
                scr, c = cv
                ps = AP(tensor=scr.tensor, offset=scr.offset,
                        ap=[[scr.ap[0][0], COUT], [48, 8], [1, 48]])
                nc.vector.tensor_copy(out_sb[:, 8 * c : 8 * c + 8, :], ps)
                if c in OUT_DMA:
                    lo, hi, k = OUT_DMA[c]
                    dma_engs[k].dma_start(
                        out=out_d[:, lo:hi],
                        in_=out_sb[:, lo // 48 : hi // 48, :]
                        .rearrange("p a b -> p (a b)"))

            def emit_uvt_group(j, g):
                # PE transposes of compacted v chunks into a rotating PSUM
                # tile, ACT copy (converting to bf16) into uvT[j].
                t_ = alloc_scr()
                c0, c1 = 4 * g, min(4 * g + 4, NCH)
                for c in range(c0, c1):
                    tpv = AP(tensor=t_.tensor, offset=t_.offset + 12 * (c - c0),
                             ap=[[t_.ap[0][0], 128], [1, 12]]).bitcast(F32R)
                    nc.tensor.transpose(
                        tpv, uvr[j][:, 128 * c : 128 * (c + 1)], id12[:, :])
                s_ = AP(tensor=t_.tensor, offset=t_.offset,
                        ap=[[t_.ap[0][0], 128], [12, c1 - c0], [1, 12]]
                        ).bitcast(F32R)
                d_ = AP(tensor=uvT[j].tensor,
                        offset=uvT[j].offset + 13 * c0 + 1,
                        ap=[uvT[j].ap[0], [13, c1 - c0], [1, 12]])
                nc.scalar.copy(d_, s_)

            norm_st = None       # (pav, qc, j) awaiting transposes
            convA = convB = None

            for t in range(16):
                qc, j = t // 2, t % 2
                exs = []
                for r in range(NRND):
                    ps_qk = psqk.tile([128, 2, 512], F32, tag="qk")
                    for b in range(2):
                        c = 2 * r + b
                        out = ps_qk[0:128, b, 0:QC].rearrange(
                            "p (a c) -> p a c", a=QROWS)
                        nc.tensor.matmul(
                            out, ukr[j][:, 128 * c : 128 * (c + 1)],
                            q_sb[:, QROWS * qc : QROWS * (qc + 1),
                                 24 * j : 24 * j + 24],
                            start=True, stop=True)
                    ex = expool.tile([128, 2, QC], BF16, tag="ex")
                    exs.append(ex)
                    if r % 2 == 0 or r == 15:
                        nc.scalar.activation(
                            ex, ps_qk[:, :, 0:QC],
                            mybir.ActivationFunctionType.Exp)
                    else:
                        nc.vector.tensor_scalar(
                            ex[:, :, :].bitcast(I16), ps_qk[:, :, 0:QC],
                            SCH_A, SCH_B,
                            mybir.AluOpType.mult, mybir.AluOpType.add)
                    # uvT builds ride the first two slots (consumed by the
                    # burst one slot later)
                    if t == 0 and 5 <= r <= 13:
                        emit_uvt_group(0, r - 5)
                    if t == 1 and 8 <= r <= 16:
                        emit_uvt_group(1, r - 8)
                    if r == 1 and prev is not None:
                        pexs, pqc, pj = prev
                        avt = alloc_scr()
                        pav = AP(tensor=avt.tensor, offset=avt.offset,
                                 ap=[[avt.ap[0][0], 128], [13, 3], [1, 13]])
                        av_burst(pexs, pav, pj)
                        norm_chain(pav)
                        norm_st = (avt, pqc, pj)
                    if r == 3 and norm_st is not None:
                        norm_out(*norm_st)
                        norm_st = None
                    # conv chunk c reads o_pad image rows 8c-1..8c+9: its
                    # norms land by slot tA(c) = 2*((c+1)//2)+2 (tp-copy at
                    # t'+1 r3). Taps spread 3 per round to keep PE feeding.
                    if t >= 2:
                        if r == 6 and ci <= 15 \
                                and 2 * ((ci + 1) // 2) + 2 <= t:
                            convA = (alloc_scr(), ci); ci += 1
                        if convA is not None and 6 <= r <= 8:
                            conv_taps(convA, (r - 6) * 3)
                        if r == 8 and convA is not None:
                            conv_out(convA)
                            convA = None
                        if r == 11 and ci <= 15 \
                                and 2 * ((ci + 1) // 2) + 2 <= t:
                            convB = (alloc_scr(), ci); ci += 1
                        if convB is not None and 11 <= r <= 13:
                            conv_taps(convB, (r - 11) * 3)
                        if r == 13 and convB is not None:
                            conv_out(convB)
                            convB = None
                prev = (exs, qc, j)

            # drain: last slot's AV + norm, remaining conv chunks
            pexs, pqc, pj = prev
            avt = alloc_scr()
            pav = AP(tensor=avt.tensor, offset=avt.offset,
                     ap=[[avt.ap[0][0], 128], [13, 3], [1, 13]])
            av_burst(pexs, pav, pj)
            norm_chain(pav)
            norm_out(avt, pqc, pj)
            while ci <= 15:
                cv = (alloc_scr(), ci)
                for t9 in (0, 3, 6):
                    conv_taps(cv, t9)
                conv_out(cv)
                ci += 1

    nc.compile()
    return nc


_NC = None


def _get_nc():
    global _NC
    if _NC is None:
        _NC = build_nc()
    return _NC


def make_in_maps(x, wq, bq, wk, bk, wv, bv, wo):
    x = np.asarray(x, np.float32)[0]           # [64, 128, 48]
    xp = np.zeros((128, 130, 50), np.float32)
    xp[0:64, 1:129, 1:49] = x
    xp[64:128, :, 0:49] = xp[0:64, :, 1:50]    # column-shifted copy
    xp = xp.reshape(128, -1)
    s = np.float32(DPH ** -0.5)

    def taps6(w):       # [O=48, I=64, 3, 3] -> [128, 6, O] tap-paired lhsT
        t = np.transpose(w, (1, 2, 3, 0))      # [I, 3, 3, O]
        out = np.zeros((128, 6, w.shape[0]), np.float32)
        for dy in range(3):
            out[0:64, 2 * dy] = t[:, dy, 0]
            out[64:128, 2 * dy] = t[:, dy, 1]
            out[0:64, 2 * dy + 1] = t[:, dy, 2]
        return out

    wq_np = np.asarray(wq, np.float32)
    wk_np = np.asarray(wk, np.float32) * s
    wv_np = np.asarray(wv, np.float32)
    wo_np = np.asarray(wo, np.float32)
    bq_np = np.asarray(bq, np.float32)
    bk_np = np.asarray(bk, np.float32) * s
    bv_np = np.asarray(bv, np.float32)

    in_maps = []
    for h in range(8):
        c_lo = (24576 * h) // 9216
        phi = (24576 * h - 9216 * c_lo) // 64
        v_idx = PHIS.index(phi)

        wqkv = np.zeros((48, CIN, 3, 3), np.float32)
        wqkv[4 * v_idx : 4 * v_idx + 4] = wq_np[4 * h : 4 * h + 4]
        wqkv[32:36] = wk_np[c_lo : c_lo + 4]
        wqkv[36 + 4 * v_idx : 36 + 4 * v_idx + 4] = wv_np[c_lo : c_lo + 4]

        b48 = np.zeros((48,), np.float32)
        b48[4 * v_idx : 4 * v_idx + 4] = bq_np[4 * h : 4 * h + 4]
        b48[32:36] = bk_np[c_lo : c_lo + 4]
        b48[36 + 4 * v_idx : 36 + 4 * v_idx + 4] = bv_np[c_lo : c_lo + 4]

        wo_t4 = np.ascontiguousarray(
            np.transpose(wo_np[:, 4 * h : 4 * h + 4], (1, 2, 3, 0))
        ).reshape(4, -1)
        wo12 = np.zeros((12, wo_t4.shape[1]), np.float32)
        wo12[4 * v_idx : 4 * v_idx + 4] = wo_t4

        in_maps.append({
            "xp": xp,
            "wqkv_t": taps6(wqkv).reshape(128, -1),
            "b48": b48.reshape(48, 1),
            "wo_t": wo12,
            "id16": np.eye(16, dtype=np.float32),
            "id12": np.eye(12, dtype=np.float32),
            "id128": np.eye(128, dtype=np.float32),
        })
    return in_maps


def kernel(x, wq, bq, wk, bk, wv, bv, wo):
    from concourse.bass_utils import run_bass_kernel_spmd

    nc = _get_nc()
    in_maps = make_in_maps(x, wq, bq, wk, bk, wv, bv, wo)
    res = run_bass_kernel_spmd(nc, in_maps, list(range(8))).results
    out = np.zeros((COUT, H * W), np.float32)
    for m in res:
        out = out + m["out"]
    return out.reshape(1, COUT, H, W)


# revision 21
# speedup vs baseline: 1.0184x; 1.0184x over previous
"""Trainium2 Bass kernel for nn_MultiHeadAttention_75737453297867.

Sharding: one head per NeuronCore (8 heads / 8 cores). The reference's
aliased as_strided gather needs a per-core base offset 24576*h into the
flange-padded k/v storage; 24576*h mod 9216 is row-aligned (phi in
{0,48,96}), so three phi-shifted staging variants are built statically
and selection happens through host data alone (stacked conv channels,
host-built conv weights pick the active variant).

v2 pipeline: the softmax-exp stream is split across ACT (exact exp ->
bf16) and DVE (1-instruction Schraudolph exp: i16 = a*s + b bit-cast as
bf16, ~3% sawtooth that washes out in the softmax ratio).  Keys whose
gathered column lands in the zero-pad band (identical across all
channels - 1440 of 5760 per window) are compacted away; their exp(0)=1
denominator contribution is re-added as a constant.  34 key-chunks of
128 remain per window (tail chunk padded with zeroed keys).

The AV matmul is flipped: stationary = exp-score chunk [128k x 128q]
(ldweights), moving = uvT [128k x 13] (col 0 = ones = denominator), so
AV costs 13 PE rows per chunk instead of 384.  Output lands [q, 13] in
PSUM; normalization is a per-partition reciprocal + tensor_scalar mult,
then a PE transpose puts channels back on partitions for the 3x3 output
conv, which streams one q-chunk behind attention.
"""

import sys

import numpy as np

if "/opt/trn_rl_repo" not in sys.path:
    sys.path.insert(0, "/opt/trn_rl_repo")

import concourse.bass as bass
import concourse.tile as tile
from concourse import bacc
from concourse import mybir
from concourse.bass_types import AP

# Problem constants
CIN, COUT, H, W = 64, 64, 128, 48
DM, NH, DPH = 32, 8, 4
Q0, Q1, F0, F1 = 128, 24, 8, 8
M0, M1 = Q0 + 2 * F0, Q1 + 2 * F1          # 144, 40
CH = 144 * 64                              # 9216 flat padded-channel size
DST = 6144                                 # d-stride (Hp*Wp) in flat coords
PHIS = (0, 48, 96)
F32 = mybir.dt.float32
F32R = mybir.dt.float32r
BF16 = mybir.dt.bfloat16
I16 = mybir.dt.int16

# Compacted key layout: per 4-row group of the 144-row window, the keys
# whose flat column (48*m0 + 24*j + m1) mod 64 lands in the zero band
# [0,8)|[56,64) are dropped (all staged channels are zero there).
# runs[(j)] = list of (m0%4, m1_lo, m1_hi) kept.
RUNS = {
    0: [(0, 8, 40), (1, 0, 8), (1, 24, 40), (2, 0, 24), (3, 0, 40)],
    1: [(0, 0, 32), (1, 0, 40), (2, 16, 40), (3, 0, 16), (3, 32, 40)],
}
NKEEP = 4320                               # kept keys per window
NZERO = float(5760 - NKEEP)                # dropped keys -> +exp(0) each
NCH = 34                                   # key chunks of 128 (tail = 96)
UKW = NCH * 128                            # 4352 (32-col zeroed tail)
NQC = 8                                    # q chunks of 16 rows
QROWS = 16
QC = QROWS * Q1                            # 384 queries per (qc, j)
NRND = 17                                  # exp rounds per (qc, j), RPB=2

# Schraudolph exp in bf16-bits domain: i16 = A*s + B, bitcast bf16.
LOG2E = 1.4426950408889634
SCH_A = float(np.float32(128.0 * LOG2E))
SCH_B = float(np.float32(127.0 * 128.0 - 0.057985 * 128.0 + 0.5))


def _run_copies(eng, dst, src_flat, j, g0, g1):
    """Compacted window copies: 5 strided run-copies per window."""
    col = 0
    for p, lo, hi in RUNS[j]:
        ln = hi - lo
        src = AP(tensor=src_flat.tensor,
                 offset=src_flat.offset + 192 * g0 + 48 * p + 24 * j + lo,
                 ap=[src_flat.ap[0], [192, g1 - g0], [1, ln]])
        d = AP(tensor=dst.tensor, offset=dst.offset + 120 * g0 + col,
               ap=[dst.ap[0], [120, g1 - g0], [1, ln]])
        if hasattr(eng, "tensor_copy"):
            eng.tensor_copy(d, src)
        else:
            eng.copy(d, src)
        col += ln


def build_nc():
    nc = bacc.Bacc()

    xp_d = nc.dram_tensor("xp", [128, 130 * 50], F32R, kind="ExternalInput")
    wqkv_d = nc.dram_tensor("wqkv_t", [128, 6 * 48], F32R, kind="ExternalInput")
    b48_d = nc.dram_tensor("b48", [48, 1], F32, kind="ExternalInput")
    wo_d = nc.dram_tensor("wo_t", [12, 9 * 64], F32R, kind="ExternalInput")
    id16_d = nc.dram_tensor("id16", [16, 16], F32, kind="ExternalInput")
    id12_d = nc.dram_tensor("id12", [12, 12], F32R, kind="ExternalInput")
    id128_d = nc.dram_tensor("id128", [128, 128], F32R, kind="ExternalInput")
    out_d = nc.dram_tensor("out", [COUT, H * W], F32, kind="ExternalOutput")

    from contextlib import ExitStack

    with tile.TileContext(nc) as tc, ExitStack() as ctx:
        P = ctx.enter_context(tc.tile_pool(name="persist", bufs=1))
        dram = ctx.enter_context(tc.tile_pool(name="dram", bufs=1, space="DRAM"))
        ctx1 = ctx.enter_context(ExitStack())
        P1 = ctx1.enter_context(tc.tile_pool(name="phase1", bufs=1))

        # ---- input loads (xp split across two DMA lanes). Partitions
        # 64-127 hold x shifted one column left, so one matmul covers the
        # (dy,0)+(dy,1) tap pair with stacked weights (K=128) ----
        wqkv_sb = P.tile([128, 6, 48], F32R, tag="wqkv")
        nc.sync.dma_start(
            out=wqkv_sb, in_=wqkv_d[:, :].rearrange("p (t o) -> p t o", t=6)
        )
        xp_sb = P1.tile([128, 130, 50], F32R, tag="xp")
        nc.sync.dma_start(
            out=xp_sb[:, 0:45, :],
            in_=xp_d[:, 0:2250].rearrange("p (a b) -> p a b", a=45),
        )
        nc.scalar.dma_start(
            out=xp_sb[:, 45:90, :],
            in_=xp_d[:, 2250:4500].rearrange("p (a b) -> p a b", a=45),
        )
        nc.gpsimd.dma_start(
            out=xp_sb[:, 90:130, :],
            in_=xp_d[:, 4500:6500].rearrange("p (a b) -> p a b", a=40),
        )
        b48 = P.tile([48, 1], F32, tag="b48")
        nc.scalar.dma_start(out=b48, in_=b48_d[:, :])
        id16 = P.tile([16, 16], F32, tag="id16")
        nc.scalar.dma_start(out=id16, in_=id16_d[:, :])
        wo_sb = P.tile([12, 9, 64], F32R, tag="wo")
        nc.scalar.dma_start(
            out=wo_sb, in_=wo_d[:, :].rearrange("p (t o) -> p t o", t=9)
        )
        id12 = P.tile([12, 12], F32R, tag="id12")
        nc.scalar.dma_start(out=id12, in_=id12_d[:, :])
        id128 = P.tile([128, 128], F32R, tag="id128")
        nc.scalar.dma_start(out=id128, in_=id128_d[:, :])

        zero_sb = P1.tile([128, 648], F32, tag="zeros")
        nc.vector.memset(zero_sb, 0.0)
        pzero = P.tile([128, 1], F32, tag="pzero")
        nc.vector.memset(pzero, 0.0)

        # PE p-state warm-up: f32r dummy matmuls (1 cyc/row) keep PE busy
        # through the xp load so the conv starts at full clock (ramp needs
        # 3us of continuous busy; an idle gap resets to mid-clock)
        with tc.tile_pool(name="pwarm", bufs=1, space="PSUM") as pwarm:
            pw = pwarm.tile([1, 288], F32, tag="pw")
            for _ in range(30):
                nc.tensor.matmul(pw, wqkv_sb[0:1, 0, 0:1],
                                 wqkv_sb[0:1, :, :].rearrange("p a b -> p (a b)"),
                                 start=True, stop=True)

        # ---- DRAM staging buffers (3 variants x 3 channels each) ----
        kp_all = dram.tile([9, CH], F32, tag="kp")
        vp_all = dram.tile([9, CH], F32, tag="vp")
        for buf in (kp_all, vp_all):
            dst = AP(tensor=buf.tensor, offset=buf.offset,
                     ap=[[648, 128], [1, 648]])
            nc.sync.dma_start(out=dst, in_=zero_sb[:, :])

        # ---- stacked q/k/v conv: rows 0-11 = q, rows 32-47 = k4 + v12 ----
        # (kv starts at 32: engine PSUM access must be 32-partition aligned)
        q_sb = P.tile([12, 128, 48], F32R, tag="q_sb")
        kv_sb = P1.tile([16, 128, 48], F32, tag="kv_sb")
        with tc.tile_pool(name="psc", bufs=4, space="PSUM") as psc:
            for chv in range(16):
                ps = psc.tile([48, 8, 48], F32, tag="cps")
                for t in range(6):
                    dy, dx = t // 2, (t % 2) * 2
                    rhs = xp_sb[:, 8 * chv + dy : 8 * chv + dy + 8, dx : dx + 48]
                    nc.tensor.matmul(
                        ps[:, :, :], wqkv_sb[:, t, 0:48], rhs,
                        start=(t == 0), stop=(t == 5),
                    )
                nc.vector.tensor_scalar_add(
                    q_sb[:, 8 * chv : 8 * chv + 8, :], ps[0:12, :, :],
                    b48[0:12, 0:1],
                )
                nc.scalar.add(
                    kv_sb[:, 8 * chv : 8 * chv + 8, :], ps[32:48, :, :],
                    b48[32:48, 0:1],
                )

        # ---- transpose k/v to row-major [128 rows, 16 ch, 48 cols] ----
        kv_row = P1.tile([128, 16, 48], F32, tag="kv_row")
        with tc.tile_pool(name="pst", bufs=4, space="PSUM") as pst:
            for x0 in range(0, 48, 3):
                tp = pst.tile([128, 3, 16], F32, tag="tp")
                for x in range(x0, x0 + 3):
                    nc.tensor.matmul(tp[:, x - x0, :], kv_sb[:, :, x],
                                     id16[:, :], start=True, stop=True)
                tpb = tp[:, 0:1, 0:1]
                src = AP(tensor=tpb.tensor, offset=tpb.offset,
                         ap=[tpb.ap[0], [1, 16], [16, 3]])
                nc.vector.tensor_copy(kv_row[:, :, x0 : x0 + 3], src)

        # ---- phi-shifted staging writes into the padded channel images ----
        engs = [nc.sync, nc.scalar]
        ei = 0
        for buf_all, cbase in ((kp_all, lambda v: 0), (vp_all, lambda v: 4 + 4 * v)):
            for v, phi in enumerate(PHIS):
                cb = cbase(v)
                base = buf_all.offset + 3 * v * CH
                if phi == 0:
                    dst = AP(tensor=buf_all.tensor, offset=base + 8 * 64 + 8,
                             ap=[[64, 128], [CH, 3], [1, 48]])
                    engs[ei % 2].dma_start(out=dst, in_=kv_row[0:128, cb : cb + 3, :])
                    ei += 1
                else:
                    n1 = 136 - phi
                    dst1 = AP(tensor=buf_all.tensor, offset=base + 8,
                              ap=[[64, n1], [CH, 3], [1, 48]])
                    engs[ei % 2].dma_start(
                        out=dst1, in_=kv_row[phi - 8 : 128, cb : cb + 3, :])
                    ei += 1
                    n2 = phi - 8
                    dst2 = AP(tensor=buf_all.tensor,
                              offset=base + (152 - phi) * 64 + 8,
                              ap=[[64, n2], [CH, 3], [1, 48]])
                    engs[ei % 2].dma_start(
                        out=dst2, in_=kv_row[0 : phi - 8, cb + 1 : cb + 4, :])
                    ei += 1

        # ---- padded attention-output image; zero only the 1-px border ----
        o_pad = P.tile([12, 130, 50], F32R, tag="opad")
        zb = zero_sb[0:12, 0:1]
        for dst in (o_pad[:, 0, :], o_pad[:, 129, :],
                    o_pad[:, 1:129, 0], o_pad[:, 1:129, 49]):
            n = dst.free_size()
            src = AP(tensor=zb.tensor, offset=zb.offset, ap=[zb.ap[0], [0, n]])
            nc.vector.tensor_copy(dst, src)

        ctx1.close()  # free xp / kv_sb / kv_row / zeros SBUF
        ctx3 = ctx.enter_context(ExitStack())
        uvp = ctx3.enter_context(tc.tile_pool(name="uvp", bufs=2))
        ctx2 = ctx.enter_context(ExitStack())
        P2 = ctx2.enter_context(tc.tile_pool(name="phase2", bufs=1))

        # ---- flat loads. Only [0:6960] is ever read by the window views,
        # so the tiles stop there. Lane plan: uk half 1 first (feeds the
        # j=0 QK chain), then uv halves, then uk half 2 ----
        uk_flat = P2.tile([12, 6960], F32R, tag="uk")
        uv_flat = P2.tile([12, 6960], F32R, tag="uv")

        def load_flat(dst, src_all, lo, hi, eng):
            src = AP(tensor=src_all.tensor, offset=src_all.offset + lo,
                     ap=[[3 * CH, 3], [DST, 4], [1, hi - lo]])
            eng.dma_start(out=dst[:, lo:hi], in_=src.bitcast(F32R))

        load_flat(uk_flat, kp_all, 0, 1740, nc.sync)
        load_flat(uk_flat, kp_all, 1740, 3480, nc.scalar)
        load_flat(uv_flat, vp_all, 0, 3480, nc.gpsimd)
        load_flat(uk_flat, kp_all, 3480, 6960, nc.scalar)
        load_flat(uv_flat, vp_all, 3480, 6960, nc.sync)

        # ---- compacted window operands ----
        # ukr[j] [12, 4352] f32r (QK lhsT); tail cols 4320:4352 zeroed.
        # uvr[j] [12, 4352] f32r feeds the uvT transposes, then freed.
        ukr0 = P.tile([12, UKW], F32R, tag="ukr0")
        ukr1 = P.tile([12, UKW], F32R, tag="ukr1")
        ukr = [ukr0, ukr1]
        uvr = []
        for j in range(2):
            uvr_t = uvp.tile([12, UKW], F32R, tag="uvr")
            uvr.append(uvr_t)
        zb12 = pzero[0:12, 0:1]
        zbc = AP(tensor=zb12.tensor, offset=zb12.offset,
                 ap=[zb12.ap[0], [0, UKW - NKEEP]])
        for j in range(2):
            nc.vector.tensor_copy(ukr[j][:, NKEEP:UKW], zbc)
            nc.vector.tensor_copy(uvr[j][:, NKEEP:UKW], zbc)
        # j=0 on ACT/DVE in two group-halves (half 1 of the flat load
        # covers groups 0..18); j=1 on Pool (full load required anyway)
        _run_copies(nc.scalar, ukr[0], uk_flat, 0, 0, 8)
        _run_copies(nc.scalar, ukr[0], uk_flat, 0, 8, 18)
        _run_copies(nc.scalar, ukr[0], uk_flat, 0, 18, 36)
        _run_copies(nc.vector, uvr[0], uv_flat, 0, 0, 18)
        _run_copies(nc.vector, uvr[0], uv_flat, 0, 18, 36)
        _run_copies(nc.gpsimd, ukr[1], uk_flat, 1, 0, 18)
        _run_copies(nc.gpsimd, uvr[1], uv_flat, 1, 0, 18)
        _run_copies(nc.gpsimd, ukr[1], uk_flat, 1, 18, 36)
        _run_copies(nc.gpsimd, uvr[1], uv_flat, 1, 18, 36)

        # ---- uvT[j] [128, 34, 13] bf16: col 0 = ones (denominator); the
        # v-chunk transposes are emitted inside attention slots t0/t1 (uvT[j]
        # is first consumed by the AV burst one slot later) ----
        uvT = []
        for j in range(2):
            t = P.tile([128, NCH, 13], BF16, tag="uvt" + str(j))
            uvT.append(t)
            nc.vector.memset(t[:, :, 0:1], 1.0)
            # fake tail keys (96:128 of chunk 33) must not count
            nc.vector.memset(t[96:128, NCH - 1, 0:1], 0.0)

        ctx2.close()  # free uk_flat / uv_flat SBUF (uvr stays in uvp)
        PL = ctx.enter_context(tc.tile_pool(name="late", bufs=1))
        expool = ctx.enter_context(tc.tile_pool(name="expool", bufs=24))

        # ---- attention: per (qc, j) slot t: 17 QK+exp rounds (RPB=2).
        # exp alternates ACT (exact, bf16 out) / DVE (Schraudolph).
        # The AV burst for slot t-1 is emitted early in slot t; the final
        # conv streams one q-chunk behind. ----
        out_sb = PL.tile([COUT, 128, 48], F32, tag="outsb")
        dma_engs = (nc.sync, nc.gpsimd)
        OUT_DMA = {3: (0, 1536, 0), 7: (1536, 3072, 1), 11: (3072, 4608, 0),
                   12: (4608, 4992, 1), 13: (4992, 5376, 0),
                   14: (5376, 5760, 1), 15: (5760, 6144, 0)}
        den3 = PL.tile([128, 3], F32, tag="den3")
        rec3 = PL.tile([128, 3], F32, tag="rec3")
        nrm = PL.tile([128, 3, 12], F32R, tag="nrm")

        with (
            tc.tile_pool(name="psqk", bufs=4, space="PSUM") as psqk,
        ):
            prev = None          # (exs, ps_av, qc, j) of slot t-1
            ci = 0               # next final-conv chunk

            def alloc_scr():
                s = psqk.tile([128, 2, 512], F32, tag="qk")
                return s

            def av_burst(exs, ps_av, j):
                for sub in range(3):
                    for c in range(NCH):
                        ex = exs[c // 2]
                        nc.tensor.matmul(
                            ps_av[:, sub, :],
                            ex[:, c % 2, 128 * sub : 128 * (sub + 1)],
                            uvT[j][:, c, :],
                            start=(c == 0), stop=(c == NCH - 1),
                            skip_group_check=True)

            def norm_chain(ps_av):
                # DVE: den += nzero, reciprocal, per-partition-scalar mults
                src = AP(tensor=ps_av.tensor, offset=ps_av.offset,
                         ap=[[ps_av.ap[0][0], 128], [13, 3], [1, 1]])
                nc.vector.tensor_scalar(den3, src, NZERO, None,
                                        mybir.AluOpType.add)
                nc.vector.reciprocal(rec3, den3)
                for sub in range(3):
                    nc.vector.tensor_scalar(
                        nrm[:, sub, :], ps_av[:, sub, 1:13],
                        rec3[:, sub : sub + 1], None, mybir.AluOpType.mult)

            def norm_out(scr, qc, j):
                # PE transposes into the scr tile (bank0, after the av cols),
                # ACT copy into o_pad. Deferred to r3 so the PE stream never
                # blocks on the DVE norm chain.
                for sub in range(3):
                    tps = AP(tensor=scr.tensor,
                             offset=scr.offset + 40 + 128 * sub,
                             ap=[[scr.ap[0][0], 12], [1, 128]]).bitcast(F32R)
                    nc.tensor.transpose(tps, nrm[:, sub, :], id128[:, :])
                dst = o_pad[:, 1 + QROWS * qc : 1 + QROWS * (qc + 1),
                            1 + 24 * j : 25 + 24 * j]
                src_tp = AP(tensor=scr.tensor, offset=scr.offset + 40,
                            ap=[[scr.ap[0][0], 12], [24, QROWS], [1, 24]]
                            ).bitcast(F32R)
                nc.scalar.copy(dst, src_tp)

            def conv_taps(cv, t0_):
                scr, c = cv
                ps = AP(tensor=scr.tensor, offset=scr.offset,
                        ap=[[scr.ap[0][0], COUT], [48, 8], [1, 48]])
                for t9 in range(t0_, t0_ + 3):
                    dy, dx = t9 // 3, t9 % 3
                    rhs = o_pad[:, 8 * c + dy : 8 * c + dy + 8, dx : dx + 48]
                    nc.tensor.matmul(ps[:, :, :], wo_sb[:, t9, :], rhs,
                                     start=(t9 == 0), stop=(t9 == 8))

            def conv_out(cv):
                scr, c = cv
                ps = AP(tensor=scr.tensor, offset=scr.offset,
                        ap=[[scr.ap[0][0], COUT], [48, 8], [1, 48]])
                nc.vector.tensor_copy(out_sb[:, 8 * c : 8 * c + 8, :], ps)
                if c in OUT_DMA:
                    lo, hi, k = OUT_DMA[c]
                    dma_engs[k].dma_start(
                        out=out_d[:, lo:hi],
                        in_=out_sb[:, lo // 48 : hi // 48, :]
                        .rearrange("p a b -> p (a b)"))

            def emit_uvt_group(j, g):
                # PE transposes of compacted v chunks into a rotating PSUM
                # tile, ACT copy (converting to bf16) into uvT[j].
                t_ = alloc_scr()
                c0, c1 = 4 * g, min(4 * g + 4, NCH)
                for c in range(c0, c1):
                    tpv = AP(tensor=t_.tensor, offset=t_.offset + 12 * (c - c0),
                             ap=[[t_.ap[0][0], 128], [1, 12]]).bitcast(F32R)
                    nc.tensor.transpose(
                        tpv, uvr[j][:, 128 * c : 128 * (c + 1)], id12[:, :])
                s_ = AP(tensor=t_.tensor, offset=t_.offset,
                        ap=[[t_.ap[0][0], 128], [12, c1 - c0], [1, 12]]
                        ).bitcast(F32R)
                d_ = AP(tensor=uvT[j].tensor,
                        offset=uvT[j].offset + 13 * c0 + 1,
                        ap=[uvT[j].ap[0], [13, c1 - c0], [1, 12]])
                nc.scalar.copy(d_, s_)

            norm_st = None       # (pav, qc, j) awaiting transposes
            convA = convB = None

            for t in range(16):
                qc, j = t // 2, t % 2
                exs = []
                for r in range(NRND):
                    ps_qk = psqk.tile([128, 2, 512], F32, tag="qk")
                    for b in range(2):
                        c = 2 * r + b
                        out = ps_qk[0:128, b, 0:QC].rearrange(
                            "p (a c) -> p a c", a=QROWS)
                        nc.tensor.matmul(
                            out, ukr[j][:, 128 * c : 128 * (c + 1)],
                            q_sb[:, QROWS * qc : QROWS * (qc + 1),
                                 24 * j : 24 * j + 24],
                            start=True, stop=True)
                    ex = expool.tile([128, 2, QC], BF16, tag="ex")
                    exs.append(ex)
                    if r % 2 == 0 or r == 15:
                        nc.scalar.activation(
                            ex, ps_qk[:, :, 0:QC],
                            mybir.ActivationFunctionType.Exp)
                    else:
                        nc.vector.tensor_scalar(
                            ex[:, :, :].bitcast(I16), ps_qk[:, :, 0:QC],
                            SCH_A, SCH_B,
                            mybir.AluOpType.mult, mybir.AluOpType.add)
                    # uvT builds ride the first two slots (consumed by the
                    # burst one slot later)
                    if t == 0 and 5 <= r <= 13:
                        emit_uvt_group(0, r - 5)
                    if t == 1 and 8 <= r <= 16:
                        emit_uvt_group(1, r - 8)
                    if r == 1 and prev is not None:
                        pexs, pqc, pj = prev
                        avt = alloc_scr()
                        pav = AP(tensor=avt.tensor, offset=avt.offset,
                                 ap=[[avt.ap[0][0], 128], [13, 3], [1, 13]])
                        av_burst(pexs, pav, pj)
                        norm_chain(pav)
                        norm_st = (avt, pqc, pj)
                    if r == 3 and norm_st is not None:
                        norm_out(*norm_st)
                        norm_st = None
                    # conv chunk c reads o_pad image rows 8c-1..8c+9: its
                    # norms land by slot tA(c) = 2*((c+1)//2)+2 (tp-copy at
                    # t'+1 r3). Taps spread 3 per round to keep PE feeding.
                    if t >= 2:
                        if r == 6 and ci <= 15 \
                                and 2 * ((ci + 1) // 2) + 2 <= t:
                            convA = (alloc_scr(), ci); ci += 1
                        if convA is not None and 6 <= r <= 8:
                            conv_taps(convA, (r - 6) * 3)
                        if r == 8 and convA is not None:
                            conv_out(convA)
                            convA = None
                        if r == 11 and ci <= 15 \
                                and 2 * ((ci + 1) // 2) + 2 <= t:
                            convB = (alloc_scr(), ci); ci += 1
                        if convB is not None and 11 <= r <= 13:
                            conv_taps(convB, (r - 11) * 3)
                        if r == 13 and convB is not None:
                            conv_out(convB)
                            convB = None
                prev = (exs, qc, j)

            # drain: last slot's AV + norm, remaining conv chunks
            pexs, pqc, pj = prev
            avt = alloc_scr()
            pav = AP(tensor=avt.tensor, offset=avt.offset,
                     ap=[[avt.ap[0][0], 128], [13, 3], [1, 13]])
            av_burst(pexs, pav, pj)
            norm_chain(pav)
            norm_out(avt, pqc, pj)
            while ci <= 15:
                cv = (alloc_scr(), ci)
                for t9 in (0, 3, 6):
                    conv_taps(cv, t9)
                conv_out(cv)
                ci += 1

    nc.compile()
    return nc


_NC = None


def _get_nc():
    global _NC
    if _NC is None:
        _NC = build_nc()
    return _NC


def make_in_maps(x, wq, bq, wk, bk, wv, bv, wo):
    x = np.asarray(x, np.float32)[0]           # [64, 128, 48]
    xp = np.zeros((128, 130, 50), np.float32)
    xp[0:64, 1:129, 1:49] = x
    xp[64:128, :, 0:49] = xp[0:64, :, 1:50]    # column-shifted copy
    xp = xp.reshape(128, -1)
    s = np.float32(DPH ** -0.5)

    def taps6(w):       # [O=48, I=64, 3, 3] -> [128, 6, O] tap-paired lhsT
        t = np.transpose(w, (1, 2, 3, 0))      # [I, 3, 3, O]
        out = np.zeros((128, 6, w.shape[0]), np.float32)
        for dy in range(3):
            out[0:64, 2 * dy] = t[:, dy, 0]
            out[64:128, 2 * dy] = t[:, dy, 1]
            out[0:64, 2 * dy + 1] = t[:, dy, 2]
        return out

    wq_np = np.asarray(wq, np.float32)
    wk_np = np.asarray(wk, np.float32) * s
    wv_np = np.asarray(wv, np.float32)
    wo_np = np.asarray(wo, np.float32)
    bq_np = np.asarray(bq, np.float32)
    bk_np = np.asarray(bk, np.float32) * s
    bv_np = np.asarray(bv, np.float32)

    in_maps = []
    for h in range(8):
        c_lo = (24576 * h) // 9216
        phi = (24576 * h - 9216 * c_lo) // 64
        v_idx = PHIS.index(phi)

        wqkv = np.zeros((48, CIN, 3, 3), np.float32)
        wqkv[4 * v_idx : 4 * v_idx + 4] = wq_np[4 * h : 4 * h + 4]
        wqkv[32:36] = wk_np[c_lo : c_lo + 4]
        wqkv[36 + 4 * v_idx : 36 + 4 * v_idx + 4] = wv_np[c_lo : c_lo + 4]

        b48 = np.zeros((48,), np.float32)
        b48[4 * v_idx : 4 * v_idx + 4] = bq_np[4 * h : 4 * h + 4]
        b48[32:36] = bk_np[c_lo : c_lo + 4]
        b48[36 + 4 * v_idx : 36 + 4 * v_idx + 4] = bv_np[c_lo : c_lo + 4]

        wo_t4 = np.ascontiguousarray(
            np.transpose(wo_np[:, 4 * h : 4 * h + 4], (1, 2, 3, 0))
        ).reshape(4, -1)
        wo12 = np.zeros((12, wo_t4.shape[1]), np.float32)
        wo12[4 * v_idx : 4 * v_idx + 4] = wo_t4

        in_maps.append({
            "xp": xp,
            "wqkv_t": taps6(wqkv).reshape(128, -1),
            "b48": b48.reshape(48, 1),
            "wo_t": wo12,
            "id16": np.eye(16, dtype=np.float32),
            "id12": np.eye(12, dtype=np.float32),
            "id128": np.eye(128, dtype=np.float32),
        })
    return in_maps


def kernel(x, wq, bq, wk, bk, wv, bv, wo):
    from concourse.bass_utils import run_bass_kernel_spmd

    nc = _get_nc()
    in_maps = make_in_maps(x, wq, bq, wk, bk, wv, bv, wo)
    res = run_bass_kernel_spmd(nc, in_maps, list(range(8))).results
    out = np.zeros((COUT, H * W), np.float32)
    for m in res:
        out = out + m["out"]
    return out.reshape(1, COUT, H, W)


# revision 22
# speedup vs baseline: 1.0275x; 1.0090x over previous
"""Trainium2 Bass kernel for nn_MultiHeadAttention_75737453297867.

Sharding: one head per NeuronCore (8 heads / 8 cores). The reference's
aliased as_strided gather needs a per-core base offset 24576*h into the
flange-padded k/v storage; 24576*h mod 9216 is row-aligned (phi in
{0,48,96}), so three phi-shifted staging variants are built statically
and selection happens through host data alone (stacked conv channels,
host-built conv weights pick the active variant).

v2 pipeline: the softmax-exp stream is split across ACT (exact exp ->
bf16) and DVE (1-instruction Schraudolph exp: i16 = a*s + b bit-cast as
bf16, ~3% sawtooth that washes out in the softmax ratio).  Keys whose
gathered column lands in the zero-pad band (identical across all
channels - 1440 of 5760 per window) are compacted away; their exp(0)=1
denominator contribution is re-added as a constant.  34 key-chunks of
128 remain per window (tail chunk padded with zeroed keys).

The AV matmul is flipped: stationary = exp-score chunk [128k x 128q]
(ldweights), moving = uvT [128k x 13] (col 0 = ones = denominator), so
AV costs 13 PE rows per chunk instead of 384.  Output lands [q, 13] in
PSUM; normalization is a per-partition reciprocal + tensor_scalar mult,
then a PE transpose puts channels back on partitions for the 3x3 output
conv, which streams one q-chunk behind attention.
"""

import sys

import numpy as np

if "/opt/trn_rl_repo" not in sys.path:
    sys.path.insert(0, "/opt/trn_rl_repo")

import concourse.bass as bass
import concourse.tile as tile
from concourse import bacc
from concourse import mybir
from concourse.bass_types import AP

# Problem constants
CIN, COUT, H, W = 64, 64, 128, 48
DM, NH, DPH = 32, 8, 4
Q0, Q1, F0, F1 = 128, 24, 8, 8
M0, M1 = Q0 + 2 * F0, Q1 + 2 * F1          # 144, 40
CH = 144 * 64                              # 9216 flat padded-channel size
DST = 6144                                 # d-stride (Hp*Wp) in flat coords
PHIS = (0, 48, 96)
F32 = mybir.dt.float32
F32R = mybir.dt.float32r
BF16 = mybir.dt.bfloat16
I16 = mybir.dt.int16

# Compacted key layout: per 4-row group of the 144-row window, the keys
# whose flat column (48*m0 + 24*j + m1) mod 64 lands in the zero band
# [0,8)|[56,64) are dropped (all staged channels are zero there).
# runs[(j)] = list of (m0%4, m1_lo, m1_hi) kept.
RUNS = {
    0: [(0, 8, 40), (1, 0, 8), (1, 24, 40), (2, 0, 24), (3, 0, 40)],
    1: [(0, 0, 32), (1, 0, 40), (2, 16, 40), (3, 0, 16), (3, 32, 40)],
}
NKEEP = 4320                               # kept keys per window
NZERO = float(5760 - NKEEP)                # dropped keys -> +exp(0) each
NCH = 34                                   # key chunks of 128 (tail = 96)
UKW = NCH * 128                            # 4352 (32-col zeroed tail)
NQC = 8                                    # q chunks of 16 rows
QROWS = 16
QC = QROWS * Q1                            # 384 queries per (qc, j)
NRND = 17                                  # exp rounds per (qc, j), RPB=2

# Schraudolph exp in bf16-bits domain: i16 = A*s + B, bitcast bf16.
LOG2E = 1.4426950408889634
SCH_A = float(np.float32(128.0 * LOG2E))
SCH_B = float(np.float32(127.0 * 128.0 - 0.057985 * 128.0 + 0.5))


def _run_copies(eng, dst, src_flat, j, g0, g1):
    """Compacted window copies: 5 strided run-copies per window."""
    col = 0
    for p, lo, hi in RUNS[j]:
        ln = hi - lo
        src = AP(tensor=src_flat.tensor,
                 offset=src_flat.offset + 192 * g0 + 48 * p + 24 * j + lo,
                 ap=[src_flat.ap[0], [192, g1 - g0], [1, ln]])
        d = AP(tensor=dst.tensor, offset=dst.offset + 120 * g0 + col,
               ap=[dst.ap[0], [120, g1 - g0], [1, ln]])
        if hasattr(eng, "tensor_copy"):
            eng.tensor_copy(d, src)
        else:
            eng.copy(d, src)
        col += ln


def build_nc():
    nc = bacc.Bacc()

    xp_d = nc.dram_tensor("xp", [128, 130 * 50], F32R, kind="ExternalInput")
    wqkv_d = nc.dram_tensor("wqkv_t", [128, 6 * 48], F32R, kind="ExternalInput")
    b48_d = nc.dram_tensor("b48", [48, 1], F32, kind="ExternalInput")
    wo_d = nc.dram_tensor("wo_t", [12, 9 * 64], F32R, kind="ExternalInput")
    id16_d = nc.dram_tensor("id16", [16, 16], F32, kind="ExternalInput")
    id12_d = nc.dram_tensor("id12", [12, 12], F32R, kind="ExternalInput")
    id128_d = nc.dram_tensor("id128", [128, 128], F32R, kind="ExternalInput")
    out_d = nc.dram_tensor("out", [COUT, H * W], F32, kind="ExternalOutput")

    from contextlib import ExitStack

    with tile.TileContext(nc) as tc, ExitStack() as ctx:
        P = ctx.enter_context(tc.tile_pool(name="persist", bufs=1))
        dram = ctx.enter_context(tc.tile_pool(name="dram", bufs=1, space="DRAM"))
        ctx1 = ctx.enter_context(ExitStack())
        P1 = ctx1.enter_context(tc.tile_pool(name="phase1", bufs=1))

        # ---- input loads (xp split across two DMA lanes). Partitions
        # 64-127 hold x shifted one column left, so one matmul covers the
        # (dy,0)+(dy,1) tap pair with stacked weights (K=128) ----
        wqkv_sb = P.tile([128, 6, 48], F32R, tag="wqkv")
        nc.sync.dma_start(
            out=wqkv_sb, in_=wqkv_d[:, :].rearrange("p (t o) -> p t o", t=6)
        )
        xp_sb = P1.tile([128, 130, 50], F32R, tag="xp")
        nc.sync.dma_start(
            out=xp_sb[:, 0:17, :],
            in_=xp_d[:, 0:850].rearrange("p (a b) -> p a b", a=17),
        )
        nc.sync.dma_start(
            out=xp_sb[:, 17:45, :],
            in_=xp_d[:, 850:2250].rearrange("p (a b) -> p a b", a=28),
        )
        nc.scalar.dma_start(
            out=xp_sb[:, 45:90, :],
            in_=xp_d[:, 2250:4500].rearrange("p (a b) -> p a b", a=45),
        )
        nc.gpsimd.dma_start(
            out=xp_sb[:, 90:130, :],
            in_=xp_d[:, 4500:6500].rearrange("p (a b) -> p a b", a=40),
        )
        b48 = P.tile([48, 1], F32, tag="b48")
        nc.gpsimd.dma_start(out=b48, in_=b48_d[:, :])
        id16 = P.tile([16, 16], F32, tag="id16")
        nc.gpsimd.dma_start(out=id16, in_=id16_d[:, :])
        wo_sb = P.tile([12, 9, 64], F32R, tag="wo")
        nc.gpsimd.dma_start(
            out=wo_sb, in_=wo_d[:, :].rearrange("p (t o) -> p t o", t=9)
        )
        id12 = P.tile([12, 12], F32R, tag="id12")
        nc.gpsimd.dma_start(out=id12, in_=id12_d[:, :])
        id128 = P.tile([128, 128], F32R, tag="id128")
        nc.gpsimd.dma_start(out=id128, in_=id128_d[:, :])

        zero_sb = P1.tile([128, 648], F32, tag="zeros")
        nc.vector.memset(zero_sb, 0.0)
        pzero = P.tile([128, 1], F32, tag="pzero")
        nc.vector.memset(pzero, 0.0)

        # PE p-state warm-up: f32r dummy matmuls (1 cyc/row) keep PE busy
        # through the xp load so the conv starts at full clock (ramp needs
        # 3us of continuous busy; an idle gap resets to mid-clock)
        with tc.tile_pool(name="pwarm", bufs=1, space="PSUM") as pwarm:
            pw = pwarm.tile([1, 288], F32, tag="pw")
            for _ in range(14):
                nc.tensor.matmul(pw, wqkv_sb[0:1, 0, 0:1],
                                 wqkv_sb[0:1, :, :].rearrange("p a b -> p (a b)"),
                                 start=True, stop=True)

        # ---- DRAM staging buffers (3 variants x 3 channels each) ----
        kp_all = dram.tile([9, CH], F32, tag="kp")
        vp_all = dram.tile([9, CH], F32, tag="vp")
        for buf in (kp_all, vp_all):
            dst = AP(tensor=buf.tensor, offset=buf.offset,
                     ap=[[648, 128], [1, 648]])
            nc.sync.dma_start(out=dst, in_=zero_sb[:, :])

        # ---- stacked q/k/v conv: rows 0-11 = q, rows 32-47 = k4 + v12 ----
        # (kv starts at 32: engine PSUM access must be 32-partition aligned)
        q_sb = P.tile([12, 128, 48], F32R, tag="q_sb")
        kv_sb = P1.tile([16, 128, 48], F32, tag="kv_sb")
        with tc.tile_pool(name="psc", bufs=4, space="PSUM") as psc:
            for chv in range(16):
                ps = psc.tile([48, 8, 48], F32, tag="cps")
                for t in range(6):
                    dy, dx = t // 2, (t % 2) * 2
                    rhs = xp_sb[:, 8 * chv + dy : 8 * chv + dy + 8, dx : dx + 48]
                    nc.tensor.matmul(
                        ps[:, :, :], wqkv_sb[:, t, 0:48], rhs,
                        start=(t == 0), stop=(t == 5),
                    )
                nc.vector.tensor_scalar_add(
                    q_sb[:, 8 * chv : 8 * chv + 8, :], ps[0:12, :, :],
                    b48[0:12, 0:1],
                )
                nc.scalar.add(
                    kv_sb[:, 8 * chv : 8 * chv + 8, :], ps[32:48, :, :],
                    b48[32:48, 0:1],
                )

        # ---- transpose k/v to row-major [128 rows, 16 ch, 48 cols] ----
        kv_row = P1.tile([128, 16, 48], F32, tag="kv_row")
        with tc.tile_pool(name="pst", bufs=4, space="PSUM") as pst:
            for x0 in range(0, 48, 3):
                tp = pst.tile([128, 3, 16], F32, tag="tp")
                for x in range(x0, x0 + 3):
                    nc.tensor.matmul(tp[:, x - x0, :], kv_sb[:, :, x],
                                     id16[:, :], start=True, stop=True)
                tpb = tp[:, 0:1, 0:1]
                src = AP(tensor=tpb.tensor, offset=tpb.offset,
                         ap=[tpb.ap[0], [1, 16], [16, 3]])
                nc.vector.tensor_copy(kv_row[:, :, x0 : x0 + 3], src)

        # ---- phi-shifted staging writes into the padded channel images ----
        engs = [nc.sync, nc.scalar]
        ei = 0
        for buf_all, cbase in ((kp_all, lambda v: 0), (vp_all, lambda v: 4 + 4 * v)):
            for v, phi in enumerate(PHIS):
                cb = cbase(v)
                base = buf_all.offset + 3 * v * CH
                if phi == 0:
                    dst = AP(tensor=buf_all.tensor, offset=base + 8 * 64 + 8,
                             ap=[[64, 128], [CH, 3], [1, 48]])
                    engs[ei % 2].dma_start(out=dst, in_=kv_row[0:128, cb : cb + 3, :])
                    ei += 1
                else:
                    n1 = 136 - phi
                    dst1 = AP(tensor=buf_all.tensor, offset=base + 8,
                              ap=[[64, n1], [CH, 3], [1, 48]])
                    engs[ei % 2].dma_start(
                        out=dst1, in_=kv_row[phi - 8 : 128, cb : cb + 3, :])
                    ei += 1
                    n2 = phi - 8
                    dst2 = AP(tensor=buf_all.tensor,
                              offset=base + (152 - phi) * 64 + 8,
                              ap=[[64, n2], [CH, 3], [1, 48]])
                    engs[ei % 2].dma_start(
                        out=dst2, in_=kv_row[0 : phi - 8, cb + 1 : cb + 4, :])
                    ei += 1

        # ---- padded attention-output image; zero only the 1-px border ----
        o_pad = P.tile([12, 130, 50], F32R, tag="opad")
        zb = zero_sb[0:12, 0:1]
        for dst in (o_pad[:, 0, :], o_pad[:, 129, :],
                    o_pad[:, 1:129, 0], o_pad[:, 1:129, 49]):
            n = dst.free_size()
            src = AP(tensor=zb.tensor, offset=zb.offset, ap=[zb.ap[0], [0, n]])
            nc.vector.tensor_copy(dst, src)

        ctx1.close()  # free xp / kv_sb / kv_row / zeros SBUF
        ctx3 = ctx.enter_context(ExitStack())
        uvp = ctx3.enter_context(tc.tile_pool(name="uvp", bufs=2))
        ctx2 = ctx.enter_context(ExitStack())
        P2 = ctx2.enter_context(tc.tile_pool(name="phase2", bufs=1))

        # ---- flat loads. Only [0:6960] is ever read by the window views,
        # so the tiles stop there. Lane plan: uk half 1 first (feeds the
        # j=0 QK chain), then uv halves, then uk half 2 ----
        uk_flat = P2.tile([12, 6960], F32R, tag="uk")
        uv_flat = P2.tile([12, 6960], F32R, tag="uv")

        def load_flat(dst, src_all, lo, hi, eng):
            src = AP(tensor=src_all.tensor, offset=src_all.offset + lo,
                     ap=[[3 * CH, 3], [DST, 4], [1, hi - lo]])
            eng.dma_start(out=dst[:, lo:hi], in_=src.bitcast(F32R))

        load_flat(uk_flat, kp_all, 0, 1740, nc.sync)
        load_flat(uk_flat, kp_all, 1740, 3480, nc.scalar)
        load_flat(uv_flat, vp_all, 0, 3480, nc.gpsimd)
        load_flat(uk_flat, kp_all, 3480, 6960, nc.scalar)
        load_flat(uv_flat, vp_all, 3480, 6960, nc.sync)

        # ---- compacted window operands ----
        # ukr[j] [12, 4352] f32r (QK lhsT); tail cols 4320:4352 zeroed.
        # uvr[j] [12, 4352] f32r feeds the uvT transposes, then freed.
        ukr0 = P.tile([12, UKW], F32R, tag="ukr0")
        ukr1 = P.tile([12, UKW], F32R, tag="ukr1")
        ukr = [ukr0, ukr1]
        uvr = []
        for j in range(2):
            uvr_t = uvp.tile([12, UKW], F32R, tag="uvr")
            uvr.append(uvr_t)
        zb12 = pzero[0:12, 0:1]
        zbc = AP(tensor=zb12.tensor, offset=zb12.offset,
                 ap=[zb12.ap[0], [0, UKW - NKEEP]])
        for j in range(2):
            nc.vector.tensor_copy(ukr[j][:, NKEEP:UKW], zbc)
            nc.vector.tensor_copy(uvr[j][:, NKEEP:UKW], zbc)
        # j=0 on ACT/DVE in two group-halves (half 1 of the flat load
        # covers groups 0..18); j=1 on Pool (full load required anyway)
        _run_copies(nc.scalar, ukr[0], uk_flat, 0, 0, 8)
        _run_copies(nc.scalar, ukr[0], uk_flat, 0, 8, 18)
        _run_copies(nc.scalar, ukr[0], uk_flat, 0, 18, 36)
        _run_copies(nc.vector, uvr[0], uv_flat, 0, 0, 18)
        _run_copies(nc.vector, uvr[0], uv_flat, 0, 18, 36)
        _run_copies(nc.gpsimd, ukr[1], uk_flat, 1, 0, 18)
        _run_copies(nc.gpsimd, uvr[1], uv_flat, 1, 0, 18)
        _run_copies(nc.gpsimd, ukr[1], uk_flat, 1, 18, 36)
        _run_copies(nc.gpsimd, uvr[1], uv_flat, 1, 18, 36)

        # ---- uvT[j] [128, 34, 13] bf16: col 0 = ones (denominator); the
        # v-chunk transposes are emitted inside attention slots t0/t1 (uvT[j]
        # is first consumed by the AV burst one slot later) ----
        uvT = []
        for j in range(2):
            t = P.tile([128, NCH, 13], BF16, tag="uvt" + str(j))
            uvT.append(t)
            nc.vector.memset(t[:, :, 0:1], 1.0)
            # fake tail keys (96:128 of chunk 33) must not count
            nc.vector.memset(t[96:128, NCH - 1, 0:1], 0.0)

        ctx2.close()  # free uk_flat / uv_flat SBUF (uvr stays in uvp)
        PL = ctx.enter_context(tc.tile_pool(name="late", bufs=1))
        expool = ctx.enter_context(tc.tile_pool(name="expool", bufs=24))

        # ---- attention: per (qc, j) slot t: 17 QK+exp rounds (RPB=2).
        # exp alternates ACT (exact, bf16 out) / DVE (Schraudolph).
        # The AV burst for slot t-1 is emitted early in slot t; the final
        # conv streams one q-chunk behind. ----
        out_sb = PL.tile([COUT, 128, 48], F32, tag="outsb")
        dma_engs = (nc.sync, nc.gpsimd)
        OUT_DMA = {3: (0, 1536, 0), 7: (1536, 3072, 1), 11: (3072, 4608, 0),
                   12: (4608, 4992, 1), 13: (4992, 5376, 0),
                   14: (5376, 5760, 1), 15: (5760, 6144, 0)}
        den3 = PL.tile([128, 3], F32, tag="den3")
        rec3 = PL.tile([128, 3], F32, tag="rec3")
        nrm = PL.tile([128, 3, 12], F32R, tag="nrm")

        with (
            tc.tile_pool(name="psqk", bufs=4, space="PSUM") as psqk,
        ):
            prev = None          # (exs, ps_av, qc, j) of slot t-1
            ci = 0               # next final-conv chunk

            def alloc_scr():
                s = psqk.tile([128, 2, 512], F32, tag="qk")
                return s

            def av_burst(exs, ps_av, j):
                for sub in range(3):
                    for c in range(NCH):
                        ex = exs[c // 2]
                        nc.tensor.matmul(
                            ps_av[:, sub, :],
                            ex[:, c % 2, 128 * sub : 128 * (sub + 1)],
                            uvT[j][:, c, :],
                            start=(c == 0), stop=(c == NCH - 1),
                            skip_group_check=True)

            def norm_chain(ps_av):
                # DVE: den += nzero, reciprocal, per-partition-scalar mults
                src = AP(tensor=ps_av.tensor, offset=ps_av.offset,
                         ap=[[ps_av.ap[0][0], 128], [13, 3], [1, 1]])
                nc.vector.tensor_scalar(den3, src, NZERO, None,
                                        mybir.AluOpType.add)
                nc.vector.reciprocal(rec3, den3)
                for sub in range(3):
                    nc.vector.tensor_scalar(
                        nrm[:, sub, :], ps_av[:, sub, 1:13],
                        rec3[:, sub : sub + 1], None, mybir.AluOpType.mult)

            def norm_out(scr, qc, j):
                # PE transposes into the scr tile (bank0, after the av cols),
                # ACT copy into o_pad. Deferred to r3 so the PE stream never
                # blocks on the DVE norm chain.
                for sub in range(3):
                    tps = AP(tensor=scr.tensor,
                             offset=scr.offset + 40 + 128 * sub,
                             ap=[[scr.ap[0][0], 12], [1, 128]]).bitcast(F32R)
                    nc.tensor.transpose(tps, nrm[:, sub, :], id128[:, :])
                dst = o_pad[:, 1 + QROWS * qc : 1 + QROWS * (qc + 1),
                            1 + 24 * j : 25 + 24 * j]
                src_tp = AP(tensor=scr.tensor, offset=scr.offset + 40,
                            ap=[[scr.ap[0][0], 12], [24, QROWS], [1, 24]]
                            ).bitcast(F32R)
                nc.scalar.copy(dst, src_tp)

            def conv_taps(cv, t0_):
                scr, c = cv
                ps = AP(tensor=scr.tensor, offset=scr.offset,
                        ap=[[scr.ap[0][0], COUT], [48, 8], [1, 48]])
                for t9 in range(t0_, t0_ + 3):
                    dy, dx = t9 // 3, t9 % 3
                    rhs = o_pad[:, 8 * c + dy : 8 * c + dy + 8, dx : dx + 48]
                    nc.tensor.matmul(ps[:, :, :], wo_sb[:, t9, :], rhs,
                                     start=(t9 == 0), stop=(t9 == 8))

            def conv_out(cv):
                scr, c = cv
                ps = AP(tensor=scr.tensor, offset=scr.offset,
                        ap=[[scr.ap[0][0], COUT], [48, 8], [1, 48]])
                nc.vector.tensor_copy(out_sb[:, 8 * c : 8 * c + 8, :], ps)
                if c in OUT_DMA:
                    lo, hi, k = OUT_DMA[c]
                    dma_engs[k].dma_start(
                        out=out_d[:, lo:hi],
                        in_=out_sb[:, lo // 48 : hi // 48, :]
                        .rearrange("p a b -> p (a b)"))

            def emit_uvt_group(j, g):
                # PE transposes of compacted v chunks into a rotating PSUM
                # tile, ACT copy (converting to bf16) into uvT[j].
                t_ = alloc_scr()
                c0, c1 = 4 * g, min(4 * g + 4, NCH)
                for c in range(c0, c1):
                    tpv = AP(tensor=t_.tensor, offset=t_.offset + 12 * (c - c0),
                             ap=[[t_.ap[0][0], 128], [1, 12]]).bitcast(F32R)
                    nc.tensor.transpose(
                        tpv, uvr[j][:, 128 * c : 128 * (c + 1)], id12[:, :])
                s_ = AP(tensor=t_.tensor, offset=t_.offset,
                        ap=[[t_.ap[0][0], 128], [12, c1 - c0], [1, 12]]
                        ).bitcast(F32R)
                d_ = AP(tensor=uvT[j].tensor,
                        offset=uvT[j].offset + 13 * c0 + 1,
                        ap=[uvT[j].ap[0], [13, c1 - c0], [1, 12]])
                nc.scalar.copy(d_, s_)

            norm_st = None       # (pav, qc, j) awaiting transposes
            convA = convB = None

            for t in range(16):
                qc, j = t // 2, t % 2
                exs = []
                for r in range(NRND):
                    ps_qk = psqk.tile([128, 2, 512], F32, tag="qk")
                    for b in range(2):
                        c = 2 * r + b
                        out = ps_qk[0:128, b, 0:QC].rearrange(
                            "p (a c) -> p a c", a=QROWS)
                        nc.tensor.matmul(
                            out, ukr[j][:, 128 * c : 128 * (c + 1)],
                            q_sb[:, QROWS * qc : QROWS * (qc + 1),
                                 24 * j : 24 * j + 24],
                            start=True, stop=True)
                    ex = expool.tile([128, 2, QC], BF16, tag="ex")
                    exs.append(ex)
                    if r % 2 == 0 or r == 15:
                        nc.scalar.activation(
                            ex, ps_qk[:, :, 0:QC],
                            mybir.ActivationFunctionType.Exp)
                    else:
                        nc.vector.tensor_scalar(
                            ex[:, :, :].bitcast(I16), ps_qk[:, :, 0:QC],
                            SCH_A, SCH_B,
                            mybir.AluOpType.mult, mybir.AluOpType.add)
                    # uvT builds ride the first two slots (consumed by the
                    # burst one slot later)
                    if t == 0 and 5 <= r <= 13:
                        emit_uvt_group(0, r - 5)
                    if t == 1 and 8 <= r <= 16:
                        emit_uvt_group(1, r - 8)
                    if r == 1 and prev is not None:
                        pexs, pqc, pj = prev
                        avt = alloc_scr()
                        pav = AP(tensor=avt.tensor, offset=avt.offset,
                                 ap=[[avt.ap[0][0], 128], [13, 3], [1, 13]])
                        av_burst(pexs, pav, pj)
                        norm_chain(pav)
                        norm_st = (avt, pqc, pj)
                    if r == 3 and norm_st is not None:
                        norm_out(*norm_st)
                        norm_st = None
                    # conv chunk c reads o_pad image rows 8c-1..8c+9: its
                    # norms land by slot tA(c) = 2*((c+1)//2)+2 (tp-copy at
                    # t'+1 r3). Taps spread 3 per round to keep PE feeding.
                    if t >= 2:
                        if r == 6 and ci <= 15 \
                                and 2 * ((ci + 1) // 2) + 2 <= t:
                            convA = (alloc_scr(), ci); ci += 1
                        if convA is not None and 6 <= r <= 8:
                            conv_taps(convA, (r - 6) * 3)
                        if r == 8 and convA is not None:
                            conv_out(convA)
                            convA = None
                        if r == 11 and ci <= 15 \
                                and 2 * ((ci + 1) // 2) + 2 <= t:
                            convB = (alloc_scr(), ci); ci += 1
                        if convB is not None and 11 <= r <= 13:
                            conv_taps(convB, (r - 11) * 3)
                        if r == 13 and convB is not None:
                            conv_out(convB)
                            convB = None
                prev = (exs, qc, j)

            # drain: last slot's AV + norm, remaining conv chunks
            pexs, pqc, pj = prev
            avt = alloc_scr()
            pav = AP(tensor=avt.tensor, offset=avt.offset,
                     ap=[[avt.ap[0][0], 128], [13, 3], [1, 13]])
            av_burst(pexs, pav, pj)
            norm_chain(pav)
            norm_out(avt, pqc, pj)
            while ci <= 15:
                cv = (alloc_scr(), ci)
                for t9 in (0, 3, 6):
                    conv_taps(cv, t9)
                conv_out(cv)
                ci += 1

    nc.compile()
    return nc


_NC = None


def _get_nc():
    global _NC
    if _NC is None:
        _NC = build_nc()
    return _NC


def make_in_maps(x, wq, bq, wk, bk, wv, bv, wo):
    x = np.asarray(x, np.float32)[0]           # [64, 128, 48]
    xp = np.zeros((128, 130, 50), np.float32)
    xp[0:64, 1:129, 1:49] = x
    xp[64:128, :, 0:49] = xp[0:64, :, 1:50]    # column-shifted copy
    xp = xp.reshape(128, -1)
    s = np.float32(DPH ** -0.5)

    def taps6(w):       # [O=48, I=64, 3, 3] -> [128, 6, O] tap-paired lhsT
        t = np.transpose(w, (1, 2, 3, 0))      # [I, 3, 3, O]
        out = np.zeros((128, 6, w.shape[0]), np.float32)
        for dy in range(3):
            out[0:64, 2 * dy] = t[:, dy, 0]
            out[64:128, 2 * dy] = t[:, dy, 1]
            out[0:64, 2 * dy + 1] = t[:, dy, 2]
        return out

    wq_np = np.asarray(wq, np.float32)
    wk_np = np.asarray(wk, np.float32) * s
    wv_np = np.asarray(wv, np.float32)
    wo_np = np.asarray(wo, np.float32)
    bq_np = np.asarray(bq, np.float32)
    bk_np = np.asarray(bk, np.float32) * s
    bv_np = np.asarray(bv, np.float32)

    in_maps = []
    for h in range(8):
        c_lo = (24576 * h) // 9216
        phi = (24576 * h - 9216 * c_lo) // 64
        v_idx = PHIS.index(phi)

        wqkv = np.zeros((48, CIN, 3, 3), np.float32)
        wqkv[4 * v_idx : 4 * v_idx + 4] = wq_np[4 * h : 4 * h + 4]
        wqkv[32:36] = wk_np[c_lo : c_lo + 4]
        wqkv[36 + 4 * v_idx : 36 + 4 * v_idx + 4] = wv_np[c_lo : c_lo + 4]

        b48 = np.zeros((48,), np.float32)
        b48[4 * v_idx : 4 * v_idx + 4] = bq_np[4 * h : 4 * h + 4]
        b48[32:36] = bk_np[c_lo : c_lo + 4]
        b48[36 + 4 * v_idx : 36 + 4 * v_idx + 4] = bv_np[c_lo : c_lo + 4]

        wo_t4 = np.ascontiguousarray(
            np.transpose(wo_np[:, 4 * h : 4 * h + 4], (1, 2, 3, 0))
        ).reshape(4, -1)
        wo12 = np.zeros((12, wo_t4.shape[1]), np.float32)
        wo12[4 * v_idx : 4 * v_idx + 4] = wo_t4

        in_maps.append({
            "xp": xp,
            "wqkv_t": taps6(wqkv).reshape(128, -1),
            "b48": b48.reshape(48, 1),
            "wo_t": wo12,
            "id16": np.eye(16, dtype=np.float32),
            "id12": np.eye(12, dtype=np.float32),
            "id128": np.eye(128, dtype=np.float32),
        })
    return in_maps


def kernel(x, wq, bq, wk, bk, wv, bv, wo):
    from concourse.bass_utils import run_bass_kernel_spmd

    nc = _get_nc()
    in_maps = make_in_maps(x, wq, bq, wk, bk, wv, bv, wo)
    res = run_bass_kernel_spmd(nc, in_maps, list(range(8))).results
    out = np.zeros((COUT, H * W), np.float32)
    for m in res:
        out = out + m["out"]
    return out.reshape(1, COUT, H, W)
